# revision 17
# baseline (speedup 1.0000x reference)
"""Fused transformer block (LN->attn->LN->FFN, causal) on 8 trn2 NeuronCores.

Sharding: core c = 2*b + h handles batch b with head-half h (8 of 16 heads) for
the attention sub-block (tensor parallel over heads), and token-half h (1024 of
2048 tokens) for the FFN sub-block (sequence parallel). Two ReduceScatters per
core pair after out_proj (one per token quarter-pair, q-chunk order 0,2,1,3)
so the first collective overlaps the remaining attention chunks.

Layouts: activations are feature-major [feat, tok] in SBUF so every matmul
chains without transposes (out = lhsT.T @ rhs with contraction on partitions).
V is produced token-major with a ones-column appended per head so the P@V
matmul also emits the softmax denominator (psum row 64). Softmax skips the
max-subtraction (scores are O(3) at this block's scale; exp cannot overflow).
Weights and post-LN activations are bfloat16 (fp32 psum accumulation); the
residual stream, LN stats, softmax denominators and collective buffers stay
fp32. Score pairs for a head-pair share one 2-bank [128,1024] psum tile so the
mask add and exp run as single wide ops.
"""

import numpy as np
import ml_dtypes

import concourse.bass as bass
import concourse.tile as tile
from concourse import bacc, mybir
from concourse.bass_utils import run_bass_kernel_spmd

F32R = mybir.dt.float32r
F32 = mybir.dt.float32
BF16 = mybir.dt.bfloat16
AF = mybir.ActivationFunctionType

B, T, C, H = 4, 2048, 1024, 16
D = C // H            # 64
HH = H // 2           # heads per core = 8
DH = HH * D           # 512 = my heads' total dim
TH = T // 2           # 1024 = my token half
FF = 4 * C            # 4096
NEG = -30000.0
NK = C // 128         # 8 contraction tiles over C
NC_T = T // 512       # 4 q/t chunks

_cached = {}


def _ln_stats(nc, pools, xc, t_len, ones, ea, eb, eps):
    """LN stats over partitions (feature-major) for one token chunk.
    Returns (bca, bcb) psum tiles: bca = rsig broadcast, bcb = -mu broadcast."""
    px2, p3, psA, psAb = pools
    s1 = psA.tile([1, 512], F32, tag="s1")
    s2 = psA.tile([1, 512], F32, tag="s2")
    for k in range(NK):
        x2 = px2.tile([128, 512], F32R, tag="x2")
        nc.scalar.activation(x2[:, :t_len], xc[:, k, :].bitcast(F32), AF.Square)
        nc.tensor.matmul(s1[:, :t_len], ones[:, 0:1], xc[:, k, :],
                         start=(k == 0), stop=(k == NK - 1))
        nc.tensor.matmul(s2[:, :t_len], ones[:, 0:1], x2[:, :t_len],
                         start=(k == 0), stop=(k == NK - 1))
    mu = p3.tile([1, 512], F32, tag="mu")
    nc.vector.tensor_scalar_mul(mu[:, :t_len], s1[:, :t_len], 1.0 / C)
    var = p3.tile([1, 512], F32, tag="var")
    nc.vector.tensor_scalar_mul(var[:, :t_len], s2[:, :t_len], 1.0 / C)
    mu2 = p3.tile([1, 512], F32R, tag="mu2")
    nc.vector.tensor_mul(mu2[:, :t_len], mu[:, :t_len], mu[:, :t_len])
    nc.vector.tensor_sub(var[:, :t_len], var[:, :t_len], mu2[:, :t_len].bitcast(F32))
    nc.scalar.activation(var[:, :t_len], var[:, :t_len], AF.Sqrt, bias=eps[0:1, 0:1])
    stk = p3.tile([2, 512], F32, tag="stk")
    nc.vector.reciprocal_approx_fast(stk[0:1, :t_len], var[:, :t_len])     # rsig
    nmu = p3.tile([1, 512], F32, tag="nmu")
    nc.vector.tensor_scalar_mul(nmu[:, :t_len], mu[:, :t_len], -1.0)
    nc.sync.dma_start(stk[1:2, :t_len], nmu[:, :t_len])
    bca = psAb.tile([128, 512], F32, tag="bca")
    nc.tensor.matmul(bca[:, :t_len], ea[:], stk[:, :t_len],
                     start=True, stop=True)
    bcb = psAb.tile([128, 512], F32, tag="bcb")
    nc.tensor.matmul(bcb[:, :t_len], eb[:], stk[:, :t_len],
                     start=True, stop=True)
    return bca, bcb


def _build():
    nc = bacc.Bacc("TRN2", target_bir_lowering=False, debug=False,
                   enable_asserts=False, num_devices=8)

    x_d = nc.dram_tensor("x", [C, T], F32R, kind="ExternalInput").ap()
    xres_d = nc.dram_tensor("xres", [C, TH], F32R, kind="ExternalInput").ap()
    wq_d = nc.dram_tensor("wq", [C, DH], BF16, kind="ExternalInput").ap()
    wk_d = nc.dram_tensor("wk", [C, DH], BF16, kind="ExternalInput").ap()
    wv_d = nc.dram_tensor("wv", [C, DH], BF16, kind="ExternalInput").ap()
    bqk_d = nc.dram_tensor("bqk", [128, 8], F32, kind="ExternalInput").ap()
    wo_d = nc.dram_tensor("wo", [DH, C], BF16, kind="ExternalInput").ap()
    bo_d = nc.dram_tensor("bo", [128, 8], F32, kind="ExternalInput").ap()
    wfc_d = nc.dram_tensor("wfc", [C, FF], BF16, kind="ExternalInput").ap()
    bfc_d = nc.dram_tensor("bfc", [128, 32], F32, kind="ExternalInput").ap()
    wpr_d = nc.dram_tensor("wpr", [FF, C], BF16, kind="ExternalInput").ap()
    masks_d = nc.dram_tensor("masks", [4, 128, 1024], F32, kind="ExternalInput").ap()
    ones_d = nc.dram_tensor("ones", [128, 1], F32R, kind="ExternalInput").ap()
    vones_d = nc.dram_tensor("vones", [128, 16, HH, 1], BF16, kind="ExternalInput").ap()
    ea_d = nc.dram_tensor("ea", [2, 128], F32, kind="ExternalInput").ap()
    eb_d = nc.dram_tensor("eb", [2, 128], F32, kind="ExternalInput").ap()
    e2_d = nc.dram_tensor("e2", [2, 128], F32, kind="ExternalInput").ap()
    out_d = nc.dram_tensor("out", [C, TH], F32, kind="ExternalOutput").ap()

    z_dram = nc.dram_tensor("z_scratch", [2, C, TH], F32)
    zr_dram = nc.dram_tensor("zr_scratch", [C, TH], F32)

    cc_sem = nc.alloc_semaphore("cc_sem")

    with tile.TileContext(nc, trace_sim=False) as tc:
        with tc.tile_pool(name="consts", bufs=1) as consts:
            # ---- constants
            ones = consts.tile([128, 1], F32R, tag="ones")
            nc.sync.dma_start(ones[:], ones_d)
            ea = consts.tile([2, 128], F32, tag="ea")
            nc.sync.dma_start(ea[:], ea_d)
            eb = consts.tile([2, 128], F32, tag="eb")
            nc.sync.dma_start(eb[:], eb_d)
            e2 = consts.tile([2, 128], F32, tag="e2")
            nc.sync.dma_start(e2[:], e2_d)
            bqk = consts.tile([128, 8], F32, tag="bqk")
            nc.sync.dma_start(bqk[:], bqk_d)
            bo = consts.tile([128, 8], F32, tag="bo")
            nc.sync.dma_start(bo[:], bo_d)
            bfc = consts.tile([128, 32], F32, tag="bfc")
            nc.sync.dma_start(bfc[:], bfc_d)
            eps = consts.tile([1, 1], F32, tag="eps")
            nc.vector.memset(eps[:], 1e-5)

            # ---- activation slabs that live through attention
            qkv_ctx = tc.tile_pool(name="qkv_slabs", bufs=1)
            qkv_slabs = qkv_ctx.__enter__()
            qf = qkv_slabs.tile([128, 4, T], BF16, tag="qf")           # 2 MB
            kf = qkv_slabs.tile([128, 4, T], BF16, tag="kf")           # 2 MB
            vt = qkv_slabs.tile([128, 16, HH, 65], BF16, tag="vt")     # ~2.2 MB

            # ===== Phase A+B (per t-chunk): LN1 -> h1 (bf16) -> Q,K,V ===
            with (
                tc.tile_pool(name="pH", bufs=2) as pH,
                tc.tile_pool(name="pXH", bufs=2) as pXH,
                tc.tile_pool(name="px2", bufs=2) as px2,
                tc.tile_pool(name="pT1", bufs=2) as pT1,
                tc.tile_pool(name="pA3", bufs=2) as pA3,
                tc.tile_pool(name="pWqk", bufs=1) as pWqk,
                tc.tile_pool(name="psA", bufs=1, space="PSUM") as psA,
                tc.tile_pool(name="psAb", bufs=1, space="PSUM") as psAb,
                tc.tile_pool(name="psB", bufs=3, space="PSUM") as psB,
            ):
                wv = pWqk.tile([128, NK, 512], BF16, tag="wv")
                wq = pWqk.tile([128, NK, 512], BF16, tag="wq")
                wk = pWqk.tile([128, NK, 512], BF16, tag="wk")
                for t_i in range(NC_T):
                    tsl = slice(t_i * 512, (t_i + 1) * 512)
                    xc = pH.tile([128, NK, 512], F32R, tag="xc")
                    nc.sync.dma_start(
                        xc[:], x_d.rearrange("(k p) t -> p k t", p=128)[:, :, tsl])
                    if t_i == 0:
                        # weight loads queued behind the first x chunk so LN1
                        # can start as early as possible
                        nc.sync.dma_start(vt[:, :, :, 64:65], vones_d)
                        nc.sync.dma_start(
                            wv[:], wv_d.rearrange("(k p) m -> p k m", p=128))
                        nc.sync.dma_start(
                            wq[:], wq_d.rearrange("(k p) m -> p k m", p=128))
                        nc.sync.dma_start(
                            wk[:], wk_d.rearrange("(k p) m -> p k m", p=128))
                    bca, bcb = _ln_stats(nc, (px2, pA3, psA, psAb), xc, 512,
                                         ones, ea, eb, eps)
                    xh = pXH.tile([128, NK, 512], BF16, tag="xh")
                    for k in range(NK):
                        t1 = pT1.tile([128, 512], F32, tag="t1")
                        nc.vector.tensor_add(t1[:], xc[:, k, :].bitcast(F32), bcb[:])
                        nc.vector.tensor_mul(xh[:, k, :], t1[:], bca[:])
                    # xh holds h1 (bf16) for this chunk
                    for m in range(4):
                        pq = psB.tile([128, 512], F32, tag="pq")
                        for k in range(NK):
                            nc.tensor.matmul(pq[:], wq[:, k, m * 128:(m + 1) * 128],
                                             xh[:, k, :],
                                             start=(k == 0), stop=(k == NK - 1))
                        nc.scalar.activation(qf[:, m, tsl], pq[:],
                                             AF.Identity, bias=bqk[:, m:m + 1])
                        pk = psB.tile([128, 512], F32, tag="pq")
                        for k in range(NK):
                            nc.tensor.matmul(pk[:], wk[:, k, m * 128:(m + 1) * 128],
                                             xh[:, k, :],
                                             start=(k == 0), stop=(k == NK - 1))
                        nc.scalar.activation(kf[:, m, tsl], pk[:],
                                             AF.Identity, bias=bqk[:, 4 + m:5 + m])
                    for tt in range(4 * t_i, 4 * t_i + 4):
                        pv = psB.tile([128, 512], F32, tag="pq")
                        for k in range(NK):
                            nc.tensor.matmul(
                                pv[:], xh[:, k, tt * 128 - t_i * 512:(tt + 1) * 128 - t_i * 512],
                                wv[:, k, :], start=(k == 0), stop=(k == NK - 1))
                        nc.vector.tensor_copy(vt[:, tt, :, 0:64], pv[:])

            # ======== Phase C+D: attention + out_proj, per q-chunk of 512 ====
            # q-chunk order 0,2,1,3: after {0,2} the first ReduceScatter fires
            # and overlaps the attention of chunks {1,3}.
            with (
                tc.tile_pool(name="wo_pool", bufs=1) as wo_pool,
                tc.tile_pool(name="pP", bufs=6) as pP,
                tc.tile_pool(name="pR", bufs=2) as pR,
                tc.tile_pool(name="pY", bufs=2) as pY,
                tc.tile_pool(name="pZ", bufs=3) as pZ,
                tc.tile_pool(name="psS", bufs=4, space="PSUM") as psS,
                tc.tile_pool(name="psY", bufs=2, space="PSUM") as psY,
            ):
                wo = wo_pool.tile([128, 4, C], BF16, tag="wo")
                nc.sync.dma_start(wo[:], wo_d.rearrange("(k p) m -> p k m", p=128))
                masks = wo_pool.tile([128, 4, 1024], F32, tag="masks")
                nc.sync.dma_start(masks[:], masks_d.rearrange("j k q -> k j q"))
                for qc in range(NC_T):
                    qsl = slice(qc * 512, (qc + 1) * 512)
                    y_sb = pY.tile([128, 4, 512], BF16, tag="ysb")
                    for p in range(4):
                        nkc = (qc + 1) * 4
                        yab = psY.tile([128, 1024], F32, tag="yab")
                        prev = None
                        for kc in range(nkc):
                            sa = psS.tile([128, 512], F32, tag="sab")
                            sb_ = psS.tile([128, 512], F32, tag="sab")
                            ksl = slice(kc * 128, (kc + 1) * 128)
                            nc.tensor.matmul(sa[:], kf[0:64, p, ksl],
                                             qf[0:64, p, qsl],
                                             start=True, stop=True, tile_position=(0, 0))
                            nc.tensor.matmul(sb_[:], kf[64:128, p, ksl],
                                             qf[64:128, p, qsl],
                                             start=True, stop=True, tile_position=(64, 0))
                            if prev is not None:
                                # software pipeline: P@V of the previous block
                                # issues behind this block's scores so the PE
                                # never queues behind the exp it needs
                                pp, pkc = prev
                                nc.tensor.matmul(yab[0:65, 0:512], vt[:, pkc, 2 * p, :],
                                                 pp[:, 0:512],
                                                 start=(pkc == 0), stop=False)
                                nc.tensor.matmul(yab[0:65, 512:1024], vt[:, pkc, 2 * p + 1, :],
                                                 pp[:, 512:1024],
                                                 start=(pkc == 0), stop=False)
                            dj = kc - qc * 4
                            if dj >= 0:   # diagonal block: causal mask add
                                nc.vector.tensor_add(sa[:], sa[:], masks[:, dj, 0:512])
                                nc.vector.tensor_add(sb_[:], sb_[:], masks[:, dj, 512:1024])
                            pa = pP.tile([128, 1024], BF16, tag="pa")
                            nc.scalar.activation(pa[:, 0:512], sa[:], AF.Exp)
                            nc.scalar.activation(pa[:, 512:1024], sb_[:], AF.Exp)
                            prev = (pa, kc)
                        pp, pkc = prev
                        nc.tensor.matmul(yab[0:65, 0:512], vt[:, pkc, 2 * p, :],
                                         pp[:, 0:512],
                                         start=(pkc == 0), stop=True)
                        nc.tensor.matmul(yab[0:65, 512:1024], vt[:, pkc, 2 * p + 1, :],
                                         pp[:, 512:1024],
                                         start=(pkc == 0), stop=True)
                        # normalize: fast reciprocal of the two sum rows
                        # (copies to SBUF first — custom-DVE ops can't read PSUM)
                        dstk = pR.tile([2, 512], F32, tag="dstk")
                        db = pR.tile([1, 512], F32, tag="db")
                        nc.vector.tensor_copy(dstk[0:1, :], yab[64:65, 0:512])
                        nc.vector.tensor_copy(db[:], yab[64:65, 512:1024])
                        nc.sync.dma_start(dstk[1:2, :], db[:])
                        rstk = pR.tile([2, 512], F32, tag="rstk")
                        nc.vector.reciprocal_approx_fast(rstk[:], dstk[:])
                        bc = psS.tile([128, 512], F32, tag="sab")
                        nc.tensor.matmul(bc[:], e2[:], rstk[:],
                                         start=True, stop=True)
                        bc_sb = pR.tile([128, 512], F32, tag="bcsb")
                        nc.vector.tensor_copy(bc_sb[:], bc[:])
                        nc.vector.tensor_mul(y_sb[0:64, p, :], yab[0:64, 0:512],
                                             bc_sb[0:64, :])
                        nc.vector.tensor_mul(y_sb[64:128, p, :], yab[0:64, 512:1024],
                                             bc_sb[64:128, :])
                    # out_proj for this q-chunk
                    for co in range(8):
                        pz = psY.tile([128, 1024], F32, tag="yab")
                        for dsl in range(4):
                            nc.tensor.matmul(pz[:, 0:512], wo[:, dsl, co * 128:(co + 1) * 128],
                                             y_sb[:, dsl, :],
                                             start=(dsl == 0), stop=(dsl == 3))
                        zst = pZ.tile([128, 512], F32, tag="zst")
                        nc.vector.tensor_scalar_add(zst[:], pz[:, 0:512], bo[:, co:co + 1])
                        nc.sync.dma_start(
                            z_dram.ap()[qc // 2, co * 128:(co + 1) * 128,
                                        (qc % 2) * 512:(qc % 2) * 512 + 512],
                            zst[:])

            qkv_ctx.__exit__(None, None, None)

            # ==== Phase F+G: residual + LN2 -> h2; FFN; final residual =======
            # (pools created before the collective so xres/weight prefetch
            # DMAs can run during it)
            with (
                tc.tile_pool(name="ffn_slabs", bufs=1) as ffn_slabs,
                tc.tile_pool(name="pF", bufs=1) as pF,
                tc.tile_pool(name="px2f", bufs=2) as px2f,
                tc.tile_pool(name="pT1f", bufs=2) as pT1f,
                tc.tile_pool(name="pF3", bufs=2) as pF3,
                tc.tile_pool(name="pG", bufs=2) as pG,
                tc.tile_pool(name="pG2", bufs=2) as pG2,
                tc.tile_pool(name="fG", bufs=1) as fG,
                tc.tile_pool(name="oG", bufs=3) as oG,
                tc.tile_pool(name="psF", bufs=1, space="PSUM") as psF,
                tc.tile_pool(name="psFb", bufs=1, space="PSUM") as psFb,
                tc.tile_pool(name="psG", bufs=2, space="PSUM") as psG,
                tc.tile_pool(name="psG2", bufs=2, space="PSUM") as psG2,
            ):
                xm = ffn_slabs.tile([128, NK, TH], F32R, tag="xm")     # 4 MB
                h2 = ffn_slabs.tile([128, NK, TH], BF16, tag="h2")     # 2 MB

                with tc.tile_critical():
                    nc.gpsimd.collective_compute(
                        "ReduceScatter",
                        mybir.AluOpType.add,
                        replica_groups=[[0, 1], [2, 3], [4, 5], [6, 7]],
                        ins=[z_dram.ap()],
                        outs=[zr_dram.ap()],
                    ).then_inc(cc_sem)
                    nc.gpsimd.wait_ge(cc_sem, 1)

                if True:
                    for t_i in range(2):
                        tsl = slice(t_i * 512, (t_i + 1) * 512)
                        zc = pF.tile([128, NK, 512], F32, tag="zc")
                        nc.sync.dma_start(
                            zc[:], zr_dram.ap().rearrange("(k p) t -> p k t", p=128)[:, :, tsl])
                        xc = pF.tile([128, NK, 512], F32R, tag="xrc")
                        nc.sync.dma_start(
                            xc[:], xres_d.rearrange("(k p) t -> p k t", p=128)[:, :, tsl])
                        for k in range(NK):
                            nc.vector.tensor_add(xm[:, k, tsl], xc[:, k, :].bitcast(F32),
                                                 zc[:, k, :])
                        bca, bcb = _ln_stats(nc, (px2f, pF3, psF, psFb),
                                             xm[:, :, tsl], 512, ones, ea, eb, eps)
                        for k in range(NK):
                            t1 = pT1f.tile([128, 512], F32, tag="t1f")
                            nc.vector.tensor_add(t1[:], xm[:, k, tsl].bitcast(F32), bcb[:])
                            nc.vector.tensor_mul(h2[:, k, tsl], t1[:], bca[:])
                        # --- FFN for this half (weights in 1-2MB DMA batches)
                        f_sb = fG.tile([128, 32, 512], BF16, tag="fsb")   # 4 MB
                        for mb in range(8):
                            wfcb = pG.tile([128, NK, 512], BF16, tag="wfc")
                            nc.sync.dma_start(
                                wfcb[:], wfc_d.rearrange("(k p) m -> p k m", p=128)[:, :, mb * 512:(mb + 1) * 512])
                            for mi in range(4):
                                m = mb * 4 + mi
                                pf = psG.tile([128, 512], F32, tag="pf")
                                for k in range(NK):
                                    nc.tensor.matmul(pf[:], wfcb[:, k, mi * 128:(mi + 1) * 128],
                                                     h2[:, k, tsl],
                                                     start=(k == 0), stop=(k == NK - 1))
                                nc.scalar.activation(f_sb[:, m, :], pf[:], AF.Gelu,
                                                     bias=bfc[:, m:m + 1])
                        for cob in range(4):
                            wprb = pG2.tile([128, 32, 256], BF16, tag="wpr")
                            nc.sync.dma_start(
                                wprb[:], wpr_d.rearrange("(m p) c -> p m c", p=128)[:, :, cob * 256:(cob + 1) * 256])
                            for ci in range(2):
                                co = cob * 2 + ci
                                pz2 = psG2.tile([128, 512], F32, tag="pz2")
                                for m in range(32):
                                    nc.tensor.matmul(pz2[:], wprb[:, m, ci * 128:(ci + 1) * 128],
                                                     f_sb[:, m, :],
                                                     start=(m == 0), stop=(m == 31))
                                ost = oG.tile([128, 512], F32, tag="ost")
                                nc.vector.tensor_add(ost[:], pz2[:],
                                                     xm[:, co, tsl].bitcast(F32))
                                nc.sync.dma_start(out_d[co * 128:(co + 1) * 128, tsl], ost[:])

    nc.compile()
    return nc


def _host_prep(inputs):
    """Fold LN params into weights, build per-h weight sets and consts."""
    f64 = np.float64
    bf16 = ml_dtypes.bfloat16
    x = np.asarray(inputs["x"], np.float32)
    g1 = np.asarray(inputs["ln1_g"], f64); b1 = np.asarray(inputs["ln1_b"], f64)
    g2 = np.asarray(inputs["ln2_g"], f64); b2 = np.asarray(inputs["ln2_b"], f64)
    Wqkv = np.asarray(inputs["W_qkv"], f64); bqkv = np.asarray(inputs["b_qkv"], f64)
    Wo = np.asarray(inputs["W_o"], f64); bo = np.asarray(inputs["b_o"], f64)
    Wfc = np.asarray(inputs["W_fc"], f64); bfc = np.asarray(inputs["b_fc"], f64)
    Wpr = np.asarray(inputs["W_pr"], f64); bpr = np.asarray(inputs["b_pr"], np.float32)

    Wq_all = (g1[:, None] * Wqkv[:, 0:C]) / 8.0
    bq_all = (b1 @ Wqkv[:, 0:C] + bqkv[0:C]) / 8.0
    Wk_all = g1[:, None] * Wqkv[:, C:2 * C]
    bk_all = b1 @ Wqkv[:, C:2 * C] + bqkv[C:2 * C]
    Wv_all = g1[:, None] * Wqkv[:, 2 * C:3 * C]
    bv_all = b1 @ Wqkv[:, 2 * C:3 * C] + bqkv[2 * C:3 * C]
    Wfc_e = g2[:, None] * Wfc
    bfc_e = b2 @ Wfc + bfc

    masks = np.zeros((4, 128, 512), np.float32)
    for j in range(4):
        for k in range(128):
            masks[j, k, :j * 128 + k] = NEG
    masks2 = np.concatenate([masks, masks], axis=2)     # [4, 128, 1024]
    ea = np.zeros((2, 128), np.float32); ea[0, :] = 1.0
    ebm = np.zeros((2, 128), np.float32); ebm[1, :] = 1.0
    e2 = np.zeros((2, 128), np.float32); e2[0, 0:64] = 1.0; e2[1, 64:128] = 1.0
    ones = np.ones((128, 1), np.float32)

    per_h = []
    for h in range(2):
        sl = slice(h * DH, (h + 1) * DH)
        bq128 = bq_all[sl].astype(np.float32).reshape(4, 128).T     # [128,4]
        bk128 = bk_all[sl].astype(np.float32).reshape(4, 128).T
        # V bias: probs sum to 1, so P@(V+bv) = P@V + bv. Fold bv@Wo_my into
        # the out-proj bias. bo itself is added by BOTH pair cores before the
        # reduce, so each adds half.
        bo_eff = bo / 2.0 + bv_all[sl] @ Wo[sl, :]
        per_h.append(dict(
            wq=np.ascontiguousarray(Wq_all[:, sl].astype(bf16)),
            wk=np.ascontiguousarray(Wk_all[:, sl].astype(bf16)),
            wv=np.ascontiguousarray(Wv_all[:, sl].astype(bf16)),
            bqk=np.ascontiguousarray(np.concatenate([bq128, bk128], 1)),
            wo=np.ascontiguousarray(Wo[sl, :].astype(bf16)),
            bo=np.ascontiguousarray(bo_eff.astype(np.float32).reshape(8, 128).T),
            wfc=np.ascontiguousarray(Wfc_e.astype(bf16)),
            bfc=np.ascontiguousarray(bfc_e.astype(np.float32).reshape(32, 128).T),
            wpr=np.ascontiguousarray(Wpr.astype(bf16)),
        ))
    consts = dict(masks=masks2, ones=ones, ea=ea, eb=ebm, e2=e2,
                  vones=np.ones((128, 16, HH, 1), bf16))
    return x, per_h, consts, bpr


def kernel(**inputs):
    if "nc" not in _cached:
        _cached["nc"] = _build()
    nc = _cached["nc"]

    x, per_h, consts, bpr = _host_prep(inputs)

    in_maps = []
    for c in range(8):
        b, h = c // 2, c % 2
        xT = np.ascontiguousarray(x[b].T)                       # [C, T]
        xres = np.ascontiguousarray(
            x[b, h * TH:(h + 1) * TH, :].T)                     # [C, TH]
        m = dict(per_h[h])
        m["x"] = xT
        m["xres"] = xres
        m.update(consts)
        in_maps.append(m)

    res = run_bass_kernel_spmd(nc, in_maps, core_ids=list(range(8)))
    _cached["last_res"] = res

    out = np.empty((B, T, C), np.float32)
    for c in range(8):
        b, h = c // 2, c % 2
        out[b, h * TH:(h + 1) * TH, :] = res.results[c]["out"].T
    out += bpr[None, None, :]
    return out


# revision 18
# speedup vs baseline: 1.0584x; 1.0584x over previous
"""Fused transformer block (LN->attn->LN->FFN, causal) on 8 trn2 NeuronCores.

Sharding: core c = 2*b + h handles batch b with head-half h (8 of 16 heads) for
the attention sub-block (tensor parallel over heads), and token-half h (1024 of
2048 tokens) for the FFN sub-block (sequence parallel). Two ReduceScatters per
core pair after out_proj (one per token quarter-pair, q-chunk order 0,2,1,3)
so the first collective overlaps the remaining attention chunks.

Layouts: activations are feature-major [feat, tok] in SBUF so every matmul
chains without transposes (out = lhsT.T @ rhs with contraction on partitions).
V is produced token-major with a ones-column appended per head so the P@V
matmul also emits the softmax denominator (psum row 64). Softmax skips the
max-subtraction (scores are O(3) at this block's scale; exp cannot overflow).
Weights and post-LN activations are bfloat16 (fp32 psum accumulation); the
residual stream, LN stats, softmax denominators and collective buffers stay
fp32. Score pairs for a head-pair share one 2-bank [128,1024] psum tile so the
mask add and exp run as single wide ops.
"""

import numpy as np
import ml_dtypes

import concourse.bass as bass
import concourse.tile as tile
from concourse import bacc, mybir
from concourse.bass_utils import run_bass_kernel_spmd

F32R = mybir.dt.float32r
F32 = mybir.dt.float32
BF16 = mybir.dt.bfloat16
AF = mybir.ActivationFunctionType

B, T, C, H = 4, 2048, 1024, 16
D = C // H            # 64
HH = H // 2           # heads per core = 8
DH = HH * D           # 512 = my heads' total dim
TH = T // 2           # 1024 = my token half
FF = 4 * C            # 4096
NEG = -30000.0
NK = C // 128         # 8 contraction tiles over C
NC_T = T // 512       # 4 q/t chunks

_cached = {}


def _ln_stats(nc, pools, xc, t_len, ones, ea, eb, eps):
    """LN stats over partitions (feature-major) for one token chunk.
    Returns (bca, bcb) psum tiles: bca = rsig broadcast, bcb = -mu broadcast."""
    px2, p3, psA, psAb = pools
    s1 = psA.tile([1, 512], F32, tag="s1")
    s2 = psA.tile([1, 512], F32, tag="s2")
    for k in range(NK):
        x2 = px2.tile([128, 512], F32R, tag="x2")
        nc.scalar.activation(x2[:, :t_len], xc[:, k, :].bitcast(F32), AF.Square)
        nc.tensor.matmul(s1[:, :t_len], ones[:, 0:1], xc[:, k, :],
                         start=(k == 0), stop=(k == NK - 1))
        nc.tensor.matmul(s2[:, :t_len], ones[:, 0:1], x2[:, :t_len],
                         start=(k == 0), stop=(k == NK - 1))
    mu = p3.tile([1, 512], F32, tag="mu")
    nc.vector.tensor_scalar_mul(mu[:, :t_len], s1[:, :t_len], 1.0 / C)
    var = p3.tile([1, 512], F32, tag="var")
    nc.vector.tensor_scalar_mul(var[:, :t_len], s2[:, :t_len], 1.0 / C)
    mu2 = p3.tile([1, 512], F32R, tag="mu2")
    nc.vector.tensor_mul(mu2[:, :t_len], mu[:, :t_len], mu[:, :t_len])
    nc.vector.tensor_sub(var[:, :t_len], var[:, :t_len], mu2[:, :t_len].bitcast(F32))
    nc.scalar.activation(var[:, :t_len], var[:, :t_len], AF.Sqrt, bias=eps[0:1, 0:1])
    stk = p3.tile([2, 512], F32, tag="stk")
    nc.vector.reciprocal_approx_fast(stk[0:1, :t_len], var[:, :t_len])     # rsig
    nmu = p3.tile([1, 512], F32, tag="nmu")
    nc.vector.tensor_scalar_mul(nmu[:, :t_len], mu[:, :t_len], -1.0)
    nc.sync.dma_start(stk[1:2, :t_len], nmu[:, :t_len])
    bca = psAb.tile([128, 512], F32, tag="bca")
    nc.tensor.matmul(bca[:, :t_len], ea[:], stk[:, :t_len],
                     start=True, stop=True)
    bcb = psAb.tile([128, 512], F32, tag="bcb")
    nc.tensor.matmul(bcb[:, :t_len], eb[:], stk[:, :t_len],
                     start=True, stop=True)
    return bca, bcb


def _build():
    nc = bacc.Bacc("TRN2", target_bir_lowering=False, debug=False,
                   enable_asserts=False, num_devices=8)

    x_d = nc.dram_tensor("x", [C, T], F32R, kind="ExternalInput").ap()
    xres_d = nc.dram_tensor("xres", [C, TH], F32R, kind="ExternalInput").ap()
    wq_d = nc.dram_tensor("wq", [C, DH], BF16, kind="ExternalInput").ap()
    wk_d = nc.dram_tensor("wk", [C, DH], BF16, kind="ExternalInput").ap()
    wv_d = nc.dram_tensor("wv", [C, DH], BF16, kind="ExternalInput").ap()
    bqk_d = nc.dram_tensor("bqk", [128, 8], F32, kind="ExternalInput").ap()
    wo_d = nc.dram_tensor("wo", [DH, C], BF16, kind="ExternalInput").ap()
    bo_d = nc.dram_tensor("bo", [128, 8], F32, kind="ExternalInput").ap()
    wfc_d = nc.dram_tensor("wfc", [C, FF], BF16, kind="ExternalInput").ap()
    bfc_d = nc.dram_tensor("bfc", [128, 32], F32, kind="ExternalInput").ap()
    wpr_d = nc.dram_tensor("wpr", [FF, C], BF16, kind="ExternalInput").ap()
    masks_d = nc.dram_tensor("masks", [4, 128, 1024], F32, kind="ExternalInput").ap()
    ones_d = nc.dram_tensor("ones", [128, 1], F32R, kind="ExternalInput").ap()
    vones_d = nc.dram_tensor("vones", [128, 16, HH, 1], BF16, kind="ExternalInput").ap()
    ea_d = nc.dram_tensor("ea", [2, 128], F32, kind="ExternalInput").ap()
    eb_d = nc.dram_tensor("eb", [2, 128], F32, kind="ExternalInput").ap()
    e2_d = nc.dram_tensor("e2", [2, 128], F32, kind="ExternalInput").ap()
    out_d = nc.dram_tensor("out", [C, TH], F32, kind="ExternalOutput").ap()

    zA_dram = nc.dram_tensor("zA_scratch", [2, C, 512], F32)
    zB_dram = nc.dram_tensor("zB_scratch", [2, C, 512], F32)
    zrA_dram = nc.dram_tensor("zrA_scratch", [C, 512], F32)
    zrB_dram = nc.dram_tensor("zrB_scratch", [C, 512], F32)

    cc_sem = nc.alloc_semaphore("cc_sem")

    with tile.TileContext(nc, trace_sim=False) as tc:
        with tc.tile_pool(name="consts", bufs=1) as consts:
            # ---- constants
            ones = consts.tile([128, 1], F32R, tag="ones")
            nc.sync.dma_start(ones[:], ones_d)
            ea = consts.tile([2, 128], F32, tag="ea")
            nc.sync.dma_start(ea[:], ea_d)
            eb = consts.tile([2, 128], F32, tag="eb")
            nc.sync.dma_start(eb[:], eb_d)
            e2 = consts.tile([2, 128], F32, tag="e2")
            nc.sync.dma_start(e2[:], e2_d)
            bqk = consts.tile([128, 8], F32, tag="bqk")
            nc.sync.dma_start(bqk[:], bqk_d)
            bo = consts.tile([128, 8], F32, tag="bo")
            nc.sync.dma_start(bo[:], bo_d)
            bfc = consts.tile([128, 32], F32, tag="bfc")
            nc.sync.dma_start(bfc[:], bfc_d)
            eps = consts.tile([1, 1], F32, tag="eps")
            nc.vector.memset(eps[:], 1e-5)

            # ---- activation slabs that live through attention
            qkv_ctx = tc.tile_pool(name="qkv_slabs", bufs=1)
            qkv_slabs = qkv_ctx.__enter__()
            qf = qkv_slabs.tile([128, 4, T], BF16, tag="qf")           # 2 MB
            kf = qkv_slabs.tile([128, 4, T], BF16, tag="kf")           # 2 MB
            vt = qkv_slabs.tile([128, 16, HH, 65], BF16, tag="vt")     # ~2.2 MB

            # ===== Phase A+B (per t-chunk): LN1 -> h1 (bf16) -> Q,K,V ===
            with (
                tc.tile_pool(name="pH", bufs=2) as pH,
                tc.tile_pool(name="pXH", bufs=2) as pXH,
                tc.tile_pool(name="px2", bufs=2) as px2,
                tc.tile_pool(name="pT1", bufs=2) as pT1,
                tc.tile_pool(name="pA3", bufs=2) as pA3,
                tc.tile_pool(name="pWqk", bufs=1) as pWqk,
                tc.tile_pool(name="psA", bufs=1, space="PSUM") as psA,
                tc.tile_pool(name="psAb", bufs=1, space="PSUM") as psAb,
                tc.tile_pool(name="psB", bufs=4, space="PSUM") as psB,
            ):
                wv = pWqk.tile([128, NK, 512], BF16, tag="wv")
                wq = pWqk.tile([128, NK, 512], BF16, tag="wq")
                wk = pWqk.tile([128, NK, 512], BF16, tag="wk")
                for t_i in range(NC_T):
                    tsl = slice(t_i * 512, (t_i + 1) * 512)
                    xc = pH.tile([128, NK, 512], F32R, tag="xc")
                    nc.sync.dma_start(
                        xc[:], x_d.rearrange("(k p) t -> p k t", p=128)[:, :, tsl])
                    if t_i == 0:
                        # weight loads queued behind the first x chunk so LN1
                        # can start as early as possible
                        nc.sync.dma_start(vt[:, :, :, 64:65], vones_d)
                        nc.sync.dma_start(
                            wv[:], wv_d.rearrange("(k p) m -> p k m", p=128))
                        nc.sync.dma_start(
                            wq[:], wq_d.rearrange("(k p) m -> p k m", p=128))
                        nc.sync.dma_start(
                            wk[:], wk_d.rearrange("(k p) m -> p k m", p=128))
                    bca, bcb = _ln_stats(nc, (px2, pA3, psA, psAb), xc, 512,
                                         ones, ea, eb, eps)
                    xh = pXH.tile([128, NK, 512], BF16, tag="xh")
                    for k in range(NK):
                        t1 = pT1.tile([128, 512], F32, tag="t1")
                        nc.vector.tensor_add(t1[:], xc[:, k, :].bitcast(F32), bcb[:])
                        nc.vector.tensor_mul(xh[:, k, :], t1[:], bca[:])
                    # xh holds h1 (bf16) for this chunk
                    for m in range(4):
                        pq = psB.tile([128, 512], F32, tag="pq")
                        for k in range(NK):
                            nc.tensor.matmul(pq[:], wq[:, k, m * 128:(m + 1) * 128],
                                             xh[:, k, :],
                                             start=(k == 0), stop=(k == NK - 1))
                        nc.scalar.activation(qf[:, m, tsl], pq[:],
                                             AF.Identity, bias=bqk[:, m:m + 1])
                        pk = psB.tile([128, 512], F32, tag="pq")
                        for k in range(NK):
                            nc.tensor.matmul(pk[:], wk[:, k, m * 128:(m + 1) * 128],
                                             xh[:, k, :],
                                             start=(k == 0), stop=(k == NK - 1))
                        nc.scalar.activation(kf[:, m, tsl], pk[:],
                                             AF.Identity, bias=bqk[:, 4 + m:5 + m])
                    for tt in range(4 * t_i, 4 * t_i + 4):
                        pv = psB.tile([128, 512], F32, tag="pq")
                        for k in range(NK):
                            nc.tensor.matmul(
                                pv[:], xh[:, k, tt * 128 - t_i * 512:(tt + 1) * 128 - t_i * 512],
                                wv[:, k, :], start=(k == 0), stop=(k == NK - 1))
                        nc.vector.tensor_copy(vt[:, tt, :, 0:64], pv[:])

            # ======== Phase C+D: attention + out_proj, per q-chunk of 512 ====
            # q-chunk order 0,2,1,3: after {0,2} the first ReduceScatter fires
            # and overlaps the attention of chunks {1,3}.
            with (
                tc.tile_pool(name="wo_pool", bufs=1) as wo_pool,
                tc.tile_pool(name="pP", bufs=6) as pP,
                tc.tile_pool(name="pR", bufs=2) as pR,
                tc.tile_pool(name="pY", bufs=2) as pY,
                tc.tile_pool(name="pZ", bufs=3) as pZ,
                tc.tile_pool(name="psS", bufs=4, space="PSUM") as psS,
                tc.tile_pool(name="psY", bufs=2, space="PSUM") as psY,
            ):
                wo = wo_pool.tile([128, 4, C], BF16, tag="wo")
                nc.sync.dma_start(wo[:], wo_d.rearrange("(k p) m -> p k m", p=128))
                masks = wo_pool.tile([128, 4, 1024], F32, tag="masks")
                nc.sync.dma_start(masks[:], masks_d.rearrange("j k q -> k j q"))
                for qc in (0, 2, 1, 3):
                    qsl = slice(qc * 512, (qc + 1) * 512)
                    y_sb = pY.tile([128, 4, 512], BF16, tag="ysb")
                    for p in range(4):
                        nkc = (qc + 1) * 4
                        yab = psY.tile([128, 1024], F32, tag="yab")
                        prev = None
                        for kc in range(nkc):
                            sa = psS.tile([128, 512], F32, tag="sab")
                            sb_ = psS.tile([128, 512], F32, tag="sab")
                            ksl = slice(kc * 128, (kc + 1) * 128)
                            nc.tensor.matmul(sa[:], kf[0:64, p, ksl],
                                             qf[0:64, p, qsl],
                                             start=True, stop=True, tile_position=(0, 0))
                            nc.tensor.matmul(sb_[:], kf[64:128, p, ksl],
                                             qf[64:128, p, qsl],
                                             start=True, stop=True, tile_position=(64, 0))
                            if prev is not None:
                                # software pipeline: P@V of the previous block
                                # issues behind this block's scores so the PE
                                # never queues behind the exp it needs
                                pp, pkc = prev
                                nc.tensor.matmul(yab[0:65, 0:512], vt[:, pkc, 2 * p, :],
                                                 pp[:, 0:512],
                                                 start=(pkc == 0), stop=False)
                                nc.tensor.matmul(yab[0:65, 512:1024], vt[:, pkc, 2 * p + 1, :],
                                                 pp[:, 512:1024],
                                                 start=(pkc == 0), stop=False)
                            dj = kc - qc * 4
                            if dj >= 0:   # diagonal block: causal mask add
                                nc.vector.tensor_add(sa[:], sa[:], masks[:, dj, 0:512])
                                nc.vector.tensor_add(sb_[:], sb_[:], masks[:, dj, 512:1024])
                            pa = pP.tile([128, 1024], BF16, tag="pa")
                            nc.scalar.activation(pa[:, 0:512], sa[:], AF.Exp)
                            nc.scalar.activation(pa[:, 512:1024], sb_[:], AF.Exp)
                            prev = (pa, kc)
                        pp, pkc = prev
                        nc.tensor.matmul(yab[0:65, 0:512], vt[:, pkc, 2 * p, :],
                                         pp[:, 0:512],
                                         start=(pkc == 0), stop=True)
                        nc.tensor.matmul(yab[0:65, 512:1024], vt[:, pkc, 2 * p + 1, :],
                                         pp[:, 512:1024],
                                         start=(pkc == 0), stop=True)
                        # normalize: fast reciprocal of the two sum rows
                        # (copies to SBUF first — custom-DVE ops can't read PSUM)
                        dstk = pR.tile([2, 512], F32, tag="dstk")
                        db = pR.tile([1, 512], F32, tag="db")
                        nc.vector.tensor_copy(dstk[0:1, :], yab[64:65, 0:512])
                        nc.vector.tensor_copy(db[:], yab[64:65, 512:1024])
                        nc.sync.dma_start(dstk[1:2, :], db[:])
                        rstk = pR.tile([2, 512], F32, tag="rstk")
                        nc.vector.reciprocal_approx_fast(rstk[:], dstk[:])
                        bc = psS.tile([128, 512], F32, tag="sab")
                        nc.tensor.matmul(bc[:], e2[:], rstk[:],
                                         start=True, stop=True)
                        bc_sb = pR.tile([128, 512], F32, tag="bcsb")
                        nc.vector.tensor_copy(bc_sb[:], bc[:])
                        nc.vector.tensor_mul(y_sb[0:64, p, :], yab[0:64, 0:512],
                                             bc_sb[0:64, :])
                        nc.vector.tensor_mul(y_sb[64:128, p, :], yab[0:64, 512:1024],
                                             bc_sb[64:128, :])
                    # out_proj for this q-chunk
                    zdst = zA_dram if qc in (0, 2) else zB_dram
                    slot = 0 if qc < 2 else 1
                    for co in range(8):
                        pz = psY.tile([128, 1024], F32, tag="yab")
                        for dsl in range(4):
                            nc.tensor.matmul(pz[:, 0:512], wo[:, dsl, co * 128:(co + 1) * 128],
                                             y_sb[:, dsl, :],
                                             start=(dsl == 0), stop=(dsl == 3))
                        zst = pZ.tile([128, 512], F32, tag="zst")
                        nc.vector.tensor_scalar_add(zst[:], pz[:, 0:512], bo[:, co:co + 1])
                        nc.sync.dma_start(
                            zdst.ap()[slot, co * 128:(co + 1) * 128, :], zst[:])
                    if qc == 2:
                        with tc.tile_critical():
                            nc.gpsimd.collective_compute(
                                "ReduceScatter",
                                mybir.AluOpType.add,
                                replica_groups=[[0, 1], [2, 3], [4, 5], [6, 7]],
                                ins=[zA_dram.ap()],
                                outs=[zrA_dram.ap()],
                            ).then_inc(cc_sem)
                            nc.gpsimd.wait_ge(cc_sem, 1)

            qkv_ctx.__exit__(None, None, None)

            with tc.tile_critical():
                nc.gpsimd.collective_compute(
                    "ReduceScatter",
                    mybir.AluOpType.add,
                    replica_groups=[[0, 1], [2, 3], [4, 5], [6, 7]],
                    ins=[zB_dram.ap()],
                    outs=[zrB_dram.ap()],
                ).then_inc(cc_sem)
                nc.gpsimd.wait_ge(cc_sem, 2)

            # ==== Phase F+G: residual + LN2 -> h2; FFN; final residual =======
            with tc.tile_pool(name="ffn_slabs", bufs=1) as ffn_slabs:
                xm = ffn_slabs.tile([128, NK, TH], F32R, tag="xm")     # 4 MB
                h2 = ffn_slabs.tile([128, NK, TH], BF16, tag="h2")     # 2 MB
                with (
                    tc.tile_pool(name="pF", bufs=1) as pF,
                    tc.tile_pool(name="px2f", bufs=2) as px2f,
                    tc.tile_pool(name="pT1f", bufs=2) as pT1f,
                    tc.tile_pool(name="pF3", bufs=2) as pF3,
                    tc.tile_pool(name="pG", bufs=2) as pG,
                    tc.tile_pool(name="pG2", bufs=2) as pG2,
                    tc.tile_pool(name="fG", bufs=1) as fG,
                    tc.tile_pool(name="oG", bufs=3) as oG,
                    tc.tile_pool(name="psF", bufs=1, space="PSUM") as psF,
                    tc.tile_pool(name="psFb", bufs=1, space="PSUM") as psFb,
                    tc.tile_pool(name="psG", bufs=2, space="PSUM") as psG,
                    tc.tile_pool(name="psG2", bufs=2, space="PSUM") as psG2,
                ):
                    for t_i in range(2):
                        tsl = slice(t_i * 512, (t_i + 1) * 512)
                        zr = zrA_dram if t_i == 0 else zrB_dram
                        zc = pF.tile([128, NK, 512], F32, tag="zc")
                        nc.sync.dma_start(
                            zc[:], zr.ap().rearrange("(k p) t -> p k t", p=128))
                        xc = pF.tile([128, NK, 512], F32R, tag="xrc")
                        nc.sync.dma_start(
                            xc[:], xres_d.rearrange("(k p) t -> p k t", p=128)[:, :, tsl])
                        for k in range(NK):
                            nc.vector.tensor_add(xm[:, k, tsl], xc[:, k, :].bitcast(F32),
                                                 zc[:, k, :])
                        bca, bcb = _ln_stats(nc, (px2f, pF3, psF, psFb),
                                             xm[:, :, tsl], 512, ones, ea, eb, eps)
                        for k in range(NK):
                            t1 = pT1f.tile([128, 512], F32, tag="t1f")
                            nc.vector.tensor_add(t1[:], xm[:, k, tsl].bitcast(F32), bcb[:])
                            nc.vector.tensor_mul(h2[:, k, tsl], t1[:], bca[:])
                        # --- FFN for this half (weights in 1-2MB DMA batches)
                        f_sb = fG.tile([128, 32, 512], BF16, tag="fsb")   # 4 MB
                        for mb in range(8):
                            wfcb = pG.tile([128, NK, 512], BF16, tag="wfc")
                            nc.sync.dma_start(
                                wfcb[:], wfc_d.rearrange("(k p) m -> p k m", p=128)[:, :, mb * 512:(mb + 1) * 512])
                            for mi in range(4):
                                m = mb * 4 + mi
                                pf = psG.tile([128, 512], F32, tag="pf")
                                for k in range(NK):
                                    nc.tensor.matmul(pf[:], wfcb[:, k, mi * 128:(mi + 1) * 128],
                                                     h2[:, k, tsl],
                                                     start=(k == 0), stop=(k == NK - 1))
                                nc.scalar.activation(f_sb[:, m, :], pf[:], AF.Gelu,
                                                     bias=bfc[:, m:m + 1])
                        for cob in range(4):
                            wprb = pG2.tile([128, 32, 256], BF16, tag="wpr")
                            nc.sync.dma_start(
                                wprb[:], wpr_d.rearrange("(m p) c -> p m c", p=128)[:, :, cob * 256:(cob + 1) * 256])
                            for ci in range(2):
                                co = cob * 2 + ci
                                pz2 = psG2.tile([128, 512], F32, tag="pz2")
                                for m in range(32):
                                    nc.tensor.matmul(pz2[:], wprb[:, m, ci * 128:(ci + 1) * 128],
                                                     f_sb[:, m, :],
                                                     start=(m == 0), stop=(m == 31))
                                ost = oG.tile([128, 512], F32, tag="ost")
                                nc.vector.tensor_add(ost[:], pz2[:],
                                                     xm[:, co, tsl].bitcast(F32))
                                nc.sync.dma_start(out_d[co * 128:(co + 1) * 128, tsl], ost[:])

    nc.compile()
    return nc


def _host_prep(inputs):
    """Fold LN params into weights, build per-h weight sets and consts."""
    f64 = np.float64
    bf16 = ml_dtypes.bfloat16
    x = np.asarray(inputs["x"], np.float32)
    g1 = np.asarray(inputs["ln1_g"], f64); b1 = np.asarray(inputs["ln1_b"], f64)
    g2 = np.asarray(inputs["ln2_g"], f64); b2 = np.asarray(inputs["ln2_b"], f64)
    Wqkv = np.asarray(inputs["W_qkv"], f64); bqkv = np.asarray(inputs["b_qkv"], f64)
    Wo = np.asarray(inputs["W_o"], f64); bo = np.asarray(inputs["b_o"], f64)
    Wfc = np.asarray(inputs["W_fc"], f64); bfc = np.asarray(inputs["b_fc"], f64)
    Wpr = np.asarray(inputs["W_pr"], f64); bpr = np.asarray(inputs["b_pr"], np.float32)

    Wq_all = (g1[:, None] * Wqkv[:, 0:C]) / 8.0
    bq_all = (b1 @ Wqkv[:, 0:C] + bqkv[0:C]) / 8.0
    Wk_all = g1[:, None] * Wqkv[:, C:2 * C]
    bk_all = b1 @ Wqkv[:, C:2 * C] + bqkv[C:2 * C]
    Wv_all = g1[:, None] * Wqkv[:, 2 * C:3 * C]
    bv_all = b1 @ Wqkv[:, 2 * C:3 * C] + bqkv[2 * C:3 * C]
    Wfc_e = g2[:, None] * Wfc
    bfc_e = b2 @ Wfc + bfc

    masks = np.zeros((4, 128, 512), np.float32)
    for j in range(4):
        for k in range(128):
            masks[j, k, :j * 128 + k] = NEG
    masks2 = np.concatenate([masks, masks], axis=2)     # [4, 128, 1024]
    ea = np.zeros((2, 128), np.float32); ea[0, :] = 1.0
    ebm = np.zeros((2, 128), np.float32); ebm[1, :] = 1.0
    e2 = np.zeros((2, 128), np.float32); e2[0, 0:64] = 1.0; e2[1, 64:128] = 1.0
    ones = np.ones((128, 1), np.float32)

    per_h = []
    for h in range(2):
        sl = slice(h * DH, (h + 1) * DH)
        bq128 = bq_all[sl].astype(np.float32).reshape(4, 128).T     # [128,4]
        bk128 = bk_all[sl].astype(np.float32).reshape(4, 128).T
        # V bias: probs sum to 1, so P@(V+bv) = P@V + bv. Fold bv@Wo_my into
        # the out-proj bias. bo itself is added by BOTH pair cores before the
        # reduce, so each adds half.
        bo_eff = bo / 2.0 + bv_all[sl] @ Wo[sl, :]
        per_h.append(dict(
            wq=np.ascontiguousarray(Wq_all[:, sl].astype(bf16)),
            wk=np.ascontiguousarray(Wk_all[:, sl].astype(bf16)),
            wv=np.ascontiguousarray(Wv_all[:, sl].astype(bf16)),
            bqk=np.ascontiguousarray(np.concatenate([bq128, bk128], 1)),
            wo=np.ascontiguousarray(Wo[sl, :].astype(bf16)),
            bo=np.ascontiguousarray(bo_eff.astype(np.float32).reshape(8, 128).T),
            wfc=np.ascontiguousarray(Wfc_e.astype(bf16)),
            bfc=np.ascontiguousarray(bfc_e.astype(np.float32).reshape(32, 128).T),
            wpr=np.ascontiguousarray(Wpr.astype(bf16)),
        ))
    consts = dict(masks=masks2, ones=ones, ea=ea, eb=ebm, e2=e2,
                  vones=np.ones((128, 16, HH, 1), bf16))
    return x, per_h, consts, bpr


def kernel(**inputs):
    if "nc" not in _cached:
        _cached["nc"] = _build()
    nc = _cached["nc"]

    x, per_h, consts, bpr = _host_prep(inputs)

    in_maps = []
    for c in range(8):
        b, h = c // 2, c % 2
        xT = np.ascontiguousarray(x[b].T)                       # [C, T]
        xres = np.ascontiguousarray(
            x[b, h * TH:(h + 1) * TH, :].T)                     # [C, TH]
        m = dict(per_h[h])
        m["x"] = xT
        m["xres"] = xres
        m.update(consts)
        in_maps.append(m)

    res = run_bass_kernel_spmd(nc, in_maps, core_ids=list(range(8)))
    _cached["last_res"] = res

    out = np.empty((B, T, C), np.float32)
    for c in range(8):
        b, h = c // 2, c % 2
        out[b, h * TH:(h + 1) * TH, :] = res.results[c]["out"].T
    out += bpr[None, None, :]
    return out
